# revision 5
# baseline (speedup 1.0000x reference)
"""Trainium2 Bass kernel for nn_CustomLoss_50843822850472.

Computes, for L2-normalized rows f of `features` [8192, 128]:
    sim = f @ f.T                      (diagonal excluded)
    e   = exp(sim / TAU)
    P_i = sum_j e_ij over {sim_ij >= alpha, j != i}   (positive mass)
    S_i = sum_j e_ij over {j != i}                    (total mass)
    loss = mean_i [ log(S_i + 2eps) - log(P_i + eps) ]   (== reference)

Sharding: rows are split across 8 NeuronCores (1024 rows/core). Each core
receives the full normalized feature matrix, pre-transposed to [D=128, N=8192]
and COLUMN-ROTATED by its row offset, so that every core's diagonal block
lands at local columns [m*128, m*128+128) of row-block m — making the
program identical (SPMD) across cores with a single static [128,128]
diagonal-mask constant.

Per core, per (row-block m in 0..7, column-chunk k in 0..3):
  - 4x matmul (float16 in, fp32 accum, K=128, M=128, N=512) -> PSUM [128, 2048]
  - (k==0) DVE adds -1e6*I to the diagonal 128 columns (kills diag via exp->0)
  - ACT computes E = exp(sim/TAU) [PSUM->SBUF] with fused row-sum -> S partial
  - DVE in-place E = (E >= beta) * E with fused row-sum -> P partial,
    where beta = exp(alpha/TAU)  (monotonic transform of `sim >= alpha`)
Final: per-row P, S are reduced on-chip, DMA'd out, and the scalar mean loss
is assembled on host in float64.
"""
import sys

sys.path.insert(0, "/opt/trn_rl_repo")

import numpy as np

TAU = 0.07
EPS = 1e-10
BIG = 1e6

N = 8192
D = 128
NCORES = 8
R = N // NCORES          # rows per core
NBLK = R // 128          # row blocks per core
CHUNK = 2048             # columns per PSUM chunk (4 banks)
NCHUNK = N // CHUNK

_CACHE = {}
LAST_RESULT = None
PROFILE = False


def _build(beta: float):
    import concourse.mybir as mybir
    from concourse import bacc, tile

    f32 = mybir.dt.float32
    f16 = mybir.dt.float16
    Alu = mybir.AluOpType

    nc = bacc.Bacc(
        "TRN2", target_bir_lowering=False, debug=False, num_devices=NCORES
    )
    ft_d = nc.dram_tensor("ft", [128, N], f16, kind="ExternalInput")
    negI_d = nc.dram_tensor("negI", [128, 128], f32, kind="ExternalInput")
    out_d = nc.dram_tensor("outPS", [128, 2 * NBLK], f32, kind="ExternalOutput")

    with tile.TileContext(nc) as tc:
        with (
            tc.tile_pool(name="sb", bufs=1) as sb,
            tc.tile_pool(name="ep", bufs=2) as ep,
            tc.tile_pool(name="pp", bufs=2, space="PSUM") as pp,
        ):
            ft = sb.tile([128, N], f16)
            for k in range(NCHUNK):
                nc.sync.dma_start(
                    ft[:, k * CHUNK:(k + 1) * CHUNK],
                    ft_d[:, k * CHUNK:(k + 1) * CHUNK],
                )
            negI = sb.tile([128, 128], f32)
            nc.sync.dma_start(negI[:], negI_d[:])

            S32 = sb.tile([128, NBLK * NCHUNK], f32)
            P32 = sb.tile([128, NBLK * NCHUNK], f32)

            for k in range(NCHUNK):
                for m in range(NBLK):
                    ps = pp.tile([128, CHUNK], f32)
                    for q in range(CHUNK // 512):
                        nc.tensor.matmul(
                            ps[:, q * 512:(q + 1) * 512],
                            lhsT=ft[:, m * 128:(m + 1) * 128],
                            rhs=ft[:, k * CHUNK + q * 512:k * CHUNK + (q + 1) * 512],
                            start=True,
                            stop=True,
                        )
                    if k == 0:
                        d0 = m * 128
                        nc.vector.tensor_tensor(
                            ps[:, d0:d0 + 128], ps[:, d0:d0 + 128], negI[:], Alu.add
                        )
                    col = m * NCHUNK + k
                    E = ep.tile([128, CHUNK], f32)
                    nc.scalar.activation(
                        E[:], ps[:], mybir.ActivationFunctionType.Exp,
                        scale=float(1.0 / TAU),
                        accum_out=S32[:, col:col + 1],
                    )
                    nc.vector.scalar_tensor_tensor(
                        out=E[:], in0=E[:], scalar=float(beta), in1=E[:],
                        op0=Alu.is_ge, op1=Alu.mult,
                        accum_out=P32[:, col:col + 1],
                    )

            out_t = sb.tile([128, 2 * NBLK], f32)
            nc.vector.reduce_sum(
                out_t[:, 0:NBLK],
                P32[:].rearrange("p (m k) -> p m k", k=NCHUNK),
                axis=mybir.AxisListType.X,
            )
            nc.vector.reduce_sum(
                out_t[:, NBLK:2 * NBLK],
                S32[:].rearrange("p (m k) -> p m k", k=NCHUNK),
                axis=mybir.AxisListType.X,
            )
            nc.sync.dma_start(out_d[:], out_t[:])
    nc.compile()
    return nc


def _prep_inputs(features: np.ndarray, alpha) -> tuple[list[dict], float]:
    feats = np.ascontiguousarray(np.asarray(features, dtype=np.float32))
    assert feats.shape == (N, D), feats.shape
    a = float(np.asarray(alpha, dtype=np.float32))

    norms = np.sqrt((feats.astype(np.float64) ** 2).sum(axis=1, keepdims=True))
    norms = np.maximum(norms, 1e-12)
    fn = (feats / norms).astype(np.float32)
    fT = np.ascontiguousarray(fn.T.astype(np.float16))  # [128, 8192] fp16

    beta = float(np.exp(np.float64(a) / TAU))
    negI = np.eye(128, dtype=np.float32) * np.float32(-BIG)

    in_maps = []
    for c in range(NCORES):
        ftc = np.ascontiguousarray(np.roll(fT, -c * R, axis=1))
        in_maps.append({"ft": ftc, "negI": negI})
    return in_maps, beta


def _assemble(results) -> np.float32:
    P = np.empty(N, np.float64)
    S = np.empty(N, np.float64)
    for c in range(NCORES):
        o = np.asarray(results[c]["outPS"], dtype=np.float64)
        P[c * R:(c + 1) * R] = o[:, 0:NBLK].T.reshape(R)
        S[c * R:(c + 1) * R] = o[:, NBLK:2 * NBLK].T.reshape(R)
    num = P + EPS
    den = num + (S - P) + EPS
    loss = np.mean(np.log(den) - np.log(num))
    return np.float32(loss)


def kernel(features, alpha):
    from concourse.bass_utils import run_bass_kernel_spmd

    global LAST_RESULT
    in_maps, beta = _prep_inputs(features, alpha)
    if beta not in _CACHE:
        _CACHE[beta] = _build(beta)
    nc = _CACHE[beta]
    res = run_bass_kernel_spmd(
        nc, in_maps, list(range(NCORES)), trace=PROFILE
    )
    LAST_RESULT = res
    return _assemble(res.results)


# revision 9
# speedup vs baseline: 1.1074x; 1.1074x over previous
"""Trainium2 Bass kernel for nn_CustomLoss_50843822850472.

Computes, for L2-normalized rows f of `features` [8192, 128]:
    sim = f @ f.T                      (diagonal excluded)
    e   = exp(sim / TAU)
    P_i = sum_j e_ij over {sim_ij >= alpha, j != i}   (positive mass)
    S_i = sum_j e_ij over {j != i}                    (total mass)
    loss = mean_i [ log(S_i + 2eps) - log(P_i + eps) ]   (== reference)

Sharding: rows are split across 8 NeuronCores (1024 rows/core). Each core
receives the full normalized feature matrix, pre-transposed to [D=128, N=8192]
(fp16) and COLUMN-ROTATED by its row offset, so that every core's diagonal
block lands at local columns [m*128, m*128+128) of row-block m — making the
program identical (SPMD) across cores with static [128,128] diagonal
constants.

Per core, per (row-block m in 0..7, column-chunk k in 0..3):
  - 4x matmul (fp16 in, fp32 accum, K=128, M=128, N=512) -> PSUM [128, 2048]
  - (k==0) an extra accumulate-matmul adds -60000*I on the diagonal 128
    columns (start=False), driving those sims to -6e4 so exp -> 0
  - ACT computes E' = exp((sim - C)/TAU) -> fp16 SBUF with fused row-sum
    accum -> S' partial (fp32).  C ~= alpha keeps E' in fp16 range and puts
    the positive threshold at exp((alpha-C)/TAU) (= 1.0 when C == alpha).
  - DVE in-place E' = (E' >= beta') * E' with fused row-sum -> P' partial.
Host rescales by exp(C/TAU) (in float64) and assembles the mean loss.

A burst of dummy matmuls at kernel start (overlapped with the input DMA)
warms the PE HAM clock gate so the real matmuls run at 2.4 GHz.
"""
import sys

sys.path.insert(0, "/opt/trn_rl_repo")

import numpy as np

TAU = 0.07
EPS = 1e-10
DIAG_NEG = -60000.0     # fp16-exact; sim + DIAG_NEG -> exp underflows to 0

N = 8192
D = 128
NCORES = 8
R = N // NCORES          # rows per core
NBLK = R // 128          # row blocks per core
CHUNK = 2048             # columns per PSUM chunk (4 banks)
NCHUNK = N // CHUNK
WARMUP_MMS = 48          # ~6us of PE busy to trip the HAM SHORT window

_CACHE = {}
LAST_RESULT = None
PROFILE = False


def _shift_center(alpha: float) -> float:
    # E' = exp((sim - C)/TAU) must fit fp16: sim <= ~1.0002, so C >= ~0.23
    # keeps max E' < 65504/some margin. C == alpha puts the threshold at 1.0.
    return float(min(max(alpha, 0.30), 1.0))


def _build(alpha: float):
    import concourse.mybir as mybir
    from concourse import bacc, tile

    f32 = mybir.dt.float32
    f16 = mybir.dt.float16
    Alu = mybir.AluOpType

    c = _shift_center(alpha)
    betap = float(np.exp((np.float64(alpha) - c) / TAU))
    bias = float(-c / TAU)

    nc = bacc.Bacc(
        "TRN2", target_bir_lowering=False, debug=False, num_devices=NCORES
    )
    ft_d = nc.dram_tensor("ft", [128, N], f16, kind="ExternalInput")
    ident_d = nc.dram_tensor("ident", [128, 128], f16, kind="ExternalInput")
    negd_d = nc.dram_tensor("negd", [128, 128], f16, kind="ExternalInput")
    out_d = nc.dram_tensor("outPS", [128, 2 * NBLK], f32, kind="ExternalOutput")

    with tile.TileContext(nc) as tc:
        with (
            tc.tile_pool(name="sb", bufs=1) as sb,
            tc.tile_pool(name="ep", bufs=2) as ep,
            tc.tile_pool(name="pp", bufs=2, space="PSUM") as pp,
        ):
            ident = sb.tile([128, 128], f16)
            nc.sync.dma_start(ident[:], ident_d[:])
            negd = sb.tile([128, 128], f16)
            nc.sync.dma_start(negd[:], negd_d[:])

            ft = sb.tile([128, N], f16)
            for k in range(NCHUNK):
                nc.sync.dma_start(
                    ft[:, k * CHUNK:(k + 1) * CHUNK],
                    ft_d[:, k * CHUNK:(k + 1) * CHUNK],
                )

            # PE warmup: dense back-to-back matmuls (result discarded) so the
            # HAM clock gate reaches 8/8 before the real matmuls start.
            warm = pp.tile([128, CHUNK], f32, tag="ps")
            for _ in range(WARMUP_MMS):
                nc.tensor.matmul(
                    warm[:, 0:128], lhsT=ident[:], rhs=ident[:],
                    start=True, stop=True,
                )

            biast = sb.tile([128, 1], f32)
            nc.vector.memset(biast[:], bias)

            S32 = sb.tile([128, NBLK * NCHUNK], f32)
            P32 = sb.tile([128, NBLK * NCHUNK], f32)

            for k in range(NCHUNK):
                for m in range(NBLK):
                    ps = pp.tile([128, CHUNK], f32, tag="ps")
                    d0 = m * 128
                    qd = d0 // 512 if k == 0 else -1
                    for q in range(CHUNK // 512):
                        nc.tensor.matmul(
                            ps[:, q * 512:(q + 1) * 512],
                            lhsT=ft[:, m * 128:(m + 1) * 128],
                            rhs=ft[:, k * CHUNK + q * 512:k * CHUNK + (q + 1) * 512],
                            start=True,
                            stop=(q != qd),
                        )
                        if q == qd:
                            # accumulate -60000 onto the diagonal 128 cols
                            nc.tensor.matmul(
                                ps[:, d0:d0 + 128],
                                lhsT=ident[:], rhs=negd[:],
                                start=False, stop=True,
                            )
                    col = m * NCHUNK + k
                    E = ep.tile([128, CHUNK], f16)
                    nc.scalar.activation(
                        E[:], ps[:], mybir.ActivationFunctionType.Exp,
                        scale=float(1.0 / TAU), bias=biast[:],
                        accum_out=S32[:, col:col + 1],
                    )
                    nc.vector.scalar_tensor_tensor(
                        out=E[:], in0=E[:], scalar=betap, in1=E[:],
                        op0=Alu.is_ge, op1=Alu.mult,
                        accum_out=P32[:, col:col + 1],
                    )

            out_t = sb.tile([128, 2 * NBLK], f32)
            nc.vector.reduce_sum(
                out_t[:, 0:NBLK],
                P32[:].rearrange("p (m k) -> p m k", k=NCHUNK),
                axis=mybir.AxisListType.X,
            )
            nc.vector.reduce_sum(
                out_t[:, NBLK:2 * NBLK],
                S32[:].rearrange("p (m k) -> p m k", k=NCHUNK),
                axis=mybir.AxisListType.X,
            )
            nc.sync.dma_start(out_d[:], out_t[:])
    nc.compile()
    return nc


def _prep_inputs(features: np.ndarray, alpha) -> tuple[list[dict], float]:
    feats = np.ascontiguousarray(np.asarray(features, dtype=np.float32))
    assert feats.shape == (N, D), feats.shape
    a = float(np.asarray(alpha, dtype=np.float32))

    norms = np.sqrt((feats.astype(np.float64) ** 2).sum(axis=1, keepdims=True))
    norms = np.maximum(norms, 1e-12)
    fn = (feats / norms).astype(np.float32)
    fT = np.ascontiguousarray(fn.T.astype(np.float16))  # [128, 8192] fp16

    ident = np.eye(128, dtype=np.float16)
    negd = (np.eye(128) * DIAG_NEG).astype(np.float16)

    in_maps = []
    for c in range(NCORES):
        ftc = np.ascontiguousarray(np.roll(fT, -c * R, axis=1))
        in_maps.append({"ft": ftc, "ident": ident, "negd": negd})
    return in_maps, a


def _assemble(results, alpha: float) -> np.float32:
    c = _shift_center(alpha)
    factor = np.exp(np.float64(c) / TAU)
    P = np.empty(N, np.float64)
    S = np.empty(N, np.float64)
    for ci in range(NCORES):
        o = np.asarray(results[ci]["outPS"], dtype=np.float64)
        P[ci * R:(ci + 1) * R] = o[:, 0:NBLK].T.reshape(R)
        S[ci * R:(ci + 1) * R] = o[:, NBLK:2 * NBLK].T.reshape(R)
    P *= factor
    S *= factor
    num = P + EPS
    den = num + (S - P) + EPS
    loss = np.mean(np.log(den) - np.log(num))
    return np.float32(loss)


def kernel(features, alpha):
    from concourse.bass_utils import run_bass_kernel_spmd

    global LAST_RESULT
    in_maps, a = _prep_inputs(features, alpha)
    if a not in _CACHE:
        _CACHE[a] = _build(a)
    nc = _CACHE[a]
    res = run_bass_kernel_spmd(
        nc, in_maps, list(range(NCORES)), trace=PROFILE
    )
    LAST_RESULT = res
    return _assemble(res.results, a)
